# revision 2
# baseline (speedup 1.0000x reference)
"""Matryoshka attention Trainium2 kernel v2: 8-core SPMD, head-parallel,
bf16, batch-software-pipelined.

24 heads across 3 tiers -> 3 heads/core; feedback folded into dense
effective K/V projections on host. All operands bf16 (f32 PSUM
accumulation); rel err ~4e-3 vs 2e-2 tolerance.

Per core, per batch b (T=1024 tokens):
  P1(b): project Q^T,K^T (dk-major) and V-hat (token-major + ones col)
         for its 3 heads from a resident bf16 x^T column block.
  P2(b): causal attention, transposed scores S^T = K Q^T, exp on ACT
         (scores bounded, no max subtraction), causal mask via mult on
         Pool (gpsimd), denominator via ones column, normalization via
         K=1 broadcast matmul of the reciprocal row.
  P3(b): partial out stripes = headout^T @ W_O rows, bf16 out DMA;
         host sums the 8 partials.
Emission interleaves P1(b+2)/P1(b+3) chunks and P3(b-1) stripes between
P2(b) score steps so the PE queue never drains; num matmuls of unit j-1
are interleaved kt-by-kt with unit j's score matmuls.
"""

import sys

if "/opt/trn_rl_repo" not in sys.path:
    sys.path.insert(0, "/opt/trn_rl_repo")

import numpy as np

import concourse.bass as bass
import concourse.tile as tile
from concourse import bacc, mybir
from concourse import bass2jax

F32 = mybir.dt.float32
F32R = mybir.dt.float32r
BF = mybir.dt.bfloat16
AF = mybir.ActivationFunctionType
MULT = mybir.AluOpType.mult

B, T, D = 4, 1024, 2048
BT = B * T
DK = 64
NH = 3
NCORES = 8
IN_OFF = [0, 256, 1024, 2048]
OUT_OFF = [0, 256, 768, 1536]
NHS = [4, 8, 12]
RANK = 8
KD = D // 128  # 16 contraction chunks


def build_nc(dbg=False, reps=1, phases=(1, 2, 3)):
    nc = bacc.Bacc("TRN2", target_bir_lowering=False, debug=False)
    xT = nc.dram_tensor("xT", [D, BT], BF, kind="ExternalInput")
    wqk = nc.dram_tensor("wqk", [D, 384], BF, kind="ExternalInput")
    wv = nc.dram_tensor("wv", [D, 192], BF, kind="ExternalInput")
    wo = nc.dram_tensor("wo", [256, D], BF, kind="ExternalInput")
    # msk: cols 0:2048 = 4 causal diagonal masks, cols 2048:2080 = ones
    msk = nc.dram_tensor("msk", [128, 2080], BF, kind="ExternalInput")
    cst = nc.dram_tensor("cst", [1, 64], F32, kind="ExternalInput")
    out = nc.dram_tensor("out", [BT, D], BF, kind="ExternalOutput")
    if dbg:
        d_qt = nc.dram_tensor("d_qt", [128, BT], BF, kind="ExternalOutput")
        d_kt = nc.dram_tensor("d_kt", [128, BT], BF, kind="ExternalOutput")
        d_qt2 = nc.dram_tensor("d_qt2", [64, BT], BF, kind="ExternalOutput")
        d_kt2 = nc.dram_tensor("d_kt2", [128, BT], BF, kind="ExternalOutput")
        d_vh = nc.dram_tensor("d_vh", [128, B * 8 * NH * 65], BF,
                              kind="ExternalOutput")
        d_hot = nc.dram_tensor("d_hot", [128, BT], BF, kind="ExternalOutput")

    with tile.TileContext(nc) as tc:
        with tc.tile_pool(name="pers", bufs=1) as pers, \
             tc.tile_pool(name="xp", bufs=2) as xp, \
             tc.tile_pool(name="esp", bufs=2) as esp, \
             tc.tile_pool(name="recp", bufs=2) as recp, \
             tc.tile_pool(name="osp", bufs=3) as osp, \
             tc.tile_pool(name="qkps", bufs=1, space="PSUM") as qkps, \
             tc.tile_pool(name="sps", bufs=3, space="PSUM") as sps, \
             tc.tile_pool(name="nps", bufs=2, space="PSUM") as nps, \
             tc.tile_pool(name="bp3", bufs=2, space="PSUM") as bp3:
            wqk_sb = pers.tile([128, KD, 384], BF)
            wv_sb = pers.tile([128, KD, 192], BF)
            wo_sb = pers.tile([128, 2, D], BF)
            mask_sb = pers.tile([128, 4, 512], BF)
            ones_sb = pers.tile([1, 64], F32R)
            QT = [pers.tile([128, T], BF, name=f"qt{b}") for b in range(B)]
            KT = [pers.tile([128, T], BF, name=f"kt{b}") for b in range(B)]
            QT2 = [pers.tile([64, T], BF, name=f"qt2_{b}") for b in range(B)]
            # KT2 rows 0:64 = K^T of head 2; rows 64:128 reused as hoT of head 2
            KT2 = [pers.tile([128, T], BF, name=f"kt2_{b}") for b in range(B)]
            VH = [pers.tile([128, 8, NH, 65], BF, name=f"vh{b}") for b in range(B)]
            HOT = [pers.tile([128, T], BF, name=f"hot{b}") for b in range(B)]

            # prelude: first x block interleaved with chunked weight loads so
            # the first matmuls start ~3us in instead of ~17us
            xs00 = pers.tile([128, KD, 512], BF, name="xs00")
            for kc in range(4):
                nc.sync.dma_start(
                    xs00[:, kc * 4:(kc + 1) * 4, :],
                    xT.ap()[kc * 512:(kc + 1) * 512, 0:512].rearrange(
                        "(k p) n -> p k n", p=128))
                nc.sync.dma_start(
                    wqk_sb[:, kc * 4:(kc + 1) * 4, :],
                    wqk.ap()[kc * 512:(kc + 1) * 512, :].rearrange(
                        "(k p) n -> p k n", p=128))
                nc.sync.dma_start(
                    wv_sb[:, kc * 4:(kc + 1) * 4, :],
                    wv.ap()[kc * 512:(kc + 1) * 512, :].rearrange(
                        "(k p) n -> p k n", p=128))
            nc.sync.dma_start(wo_sb[:], wo.ap().rearrange(
                "(k p) n -> p k n", p=128))
            nc.sync.dma_start(mask_sb[:], msk.ap()[:, 0:2048].rearrange(
                "p (i n) -> p i n", i=4))
            nc.sync.dma_start(ones_sb[:], cst.ap().bitcast(F32R))

            def emit():
                for b in range(B):
                    nc.sync.dma_start(
                        VH[b][:, :, :, 64:65],
                        msk.ap()[:, 2048:2072].rearrange(
                            "p (k h o) -> p k h o", k=8, o=1))

                # ---------- P1 chunk closures ----------
                def p1_half_closures(b, half):
                    col = b * T + half * 512
                    st = {}

                    def a_dma():
                        if b == 0 and half == 0:
                            st["xs"] = xs00
                            return
                        xs = xp.tile([128, KD, 512], BF, tag="xs", name="xs")
                        for kc in range(4):
                            nc.sync.dma_start(
                                xs[:, kc * 4:(kc + 1) * 4, :],
                                xT.ap()[kc * 512:(kc + 1) * 512,
                                        col:col + 512].rearrange(
                                    "(k p) n -> p k n", p=128))
                        st["xs"] = xs

                    def a_qk(mt):
                        def f():
                            xs = st["xs"]
                            ps = qkps.tile([128, 512], F32, tag="qk", name="qk")
                            for kd in range(KD):
                                nc.tensor.matmul(
                                    ps[:], wqk_sb[:, kd, mt * 128:(mt + 1) * 128],
                                    xs[:, kd, :], start=kd == 0, stop=kd == KD - 1)
                            dst = half * 512
                            if mt == 0:
                                nc.vector.tensor_copy(
                                    QT[b][:, dst:dst + 512], ps[:])
                            elif mt == 1:
                                nc.scalar.copy(KT[b][:, dst:dst + 512], ps[:])
                            else:
                                nc.vector.tensor_copy(
                                    QT2[b][0:64, dst:dst + 512], ps[0:64, :])
                                nc.scalar.copy(
                                    KT2[b][0:64, dst:dst + 512], ps[64:128, :])
                        return f

                    def a_v(pr):
                        def f():
                            xs = st["xs"]
                            for s2 in range(2):
                                sub = pr * 2 + s2
                                pv = nps.tile([128, 192], F32, tag="pn",
                                              name="pv")
                                for kd in range(KD):
                                    nc.tensor.matmul(
                                        pv[:],
                                        xs[:, kd, sub * 128:(sub + 1) * 128],
                                        wv_sb[:, kd, :],
                                        start=kd == 0, stop=kd == KD - 1)
                                so = half * 4 + sub
                                nc.vector.tensor_copy(
                                    VH[b][:, so, :, 0:64],
                                    pv[:].rearrange("p (h d) -> p h d", h=NH))
                        return f

                    return [a_dma, a_qk(0), a_v(0), a_qk(1), a_v(1), a_qk(2)]

                # ---------- P3 stripe closures ----------
                def p3_stripe_closures(b, s):
                    ms = slice(b * T + s * 128, b * T + (s + 1) * 128)
                    ls = slice(s * 128, (s + 1) * 128)
                    st = {}

                    def a_pair(pair):
                        def f():
                            if "osb" not in st:
                                st["osb"] = osp.tile([128, D], BF, tag="osb",
                                                     name="osb")
                            osb = st["osb"]
                            tiles = []
                            for j in range(2):
                                nt = pair * 2 + j
                                po = bp3.tile([128, 512], F32, tag="bp3",
                                              name="po")
                                nc.tensor.matmul(
                                    po[:], HOT[b][:, ls],
                                    wo_sb[:, 0, nt * 512:(nt + 1) * 512],
                                    start=True, stop=False)
                                tiles.append((nt, po))
                            for j, (nt, po) in enumerate(tiles):
                                nc.tensor.matmul(
                                    po[:], KT2[b][64:128, ls],
                                    wo_sb[64:128, 1, nt * 512:(nt + 1) * 512],
                                    start=False, stop=True)
                            for j, (nt, po) in enumerate(tiles):
                                ns = slice(nt * 512, (nt + 1) * 512)
                                if j == 0:
                                    nc.vector.tensor_copy(osb[:, ns], po[:])
                                else:
                                    nc.scalar.copy(osb[:, ns], po[:])
                            if pair == 1:
                                nc.sync.dma_start(out.ap()[ms, :], osb[:])
                        return f

                    return [a_pair(0), a_pair(1)]

                # ---------- P2 helpers ----------
                def k_lhsT(b, h, kt):
                    sl = slice(kt * 128, (kt + 1) * 128)
                    if h == 0:
                        return KT[b][0:64, sl]
                    if h == 1:
                        return KT[b][64:128, sl]
                    return KT2[b][0:64, sl]

                def q_rhs(b, h, qc):
                    sl = slice(qc * 512, (qc + 1) * 512)
                    if h == 0:
                        return QT[b][0:64, sl]
                    if h == 1:
                        return QT[b][64:128, sl]
                    return QT2[b][0:64, sl]

                def finalize(b, h, qc, pn):
                    qoff = qc * 512
                    rec = recp.tile([1, 512], F32R, tag="rec", name="rec")
                    with nc.allow_low_precision(
                            reason="softmax denominator reciprocal"):
                        nc.vector.reciprocal(rec[:], pn[64:65, :])
                    pb = bp3.tile([128, 512], F32, tag="bp3", name="pb")
                    nc.tensor.matmul(pb[0:64, :], ones_sb[:], rec[:],
                                     start=True, stop=True)
                    bc = recp.tile([64, 512], F32R, tag="bc", name="bc")
                    nc.vector.tensor_copy(bc[:], pb[0:64, :])
                    if h == 0:
                        dest = HOT[b][0:64, qoff:qoff + 512]
                    elif h == 1:
                        dest = HOT[b][64:128, qoff:qoff + 512]
                    else:
                        dest = KT2[b][64:128, qoff:qoff + 512]
                    nc.vector.tensor_tensor(dest, pn[0:64, :], bc[:], MULT)

                # ---------- pipeline ----------
                if 2 not in phases:
                    for b in range(B):
                        for c in p1_half_closures(b, 0):
                            c()
                        for c in p1_half_closures(b, 1):
                            c()
                    return

                # prologue: P1 of batches 0 and 1
                for b in (0, 1):
                    for half in (0, 1):
                        for c in p1_half_closures(b, half):
                            c()

                units = [(h, qc) for h in range(NH) for qc in range(2)]
                prev = None  # (b, h, qc, es, state)

                def emit_prev_num(pv, n):
                    """Emit up to n pending num matmuls of prev unit; returns
                    True when all emitted."""
                    (pb_, ph, pqc, pes, stt) = pv
                    pnkt = 4 * pqc + 4
                    if "pn" not in stt:
                        stt["pn"] = nps.tile([128, 512], F32, tag="pn",
                                             name="pn")
                        stt["k"] = 0
                    pn = stt["pn"]
                    while n > 0 and stt["k"] < pnkt:
                        kt = stt["k"]
                        nc.tensor.matmul(
                            pn[0:65, :], VH[pb_][:, kt, ph, :], pes[:, kt, :],
                            start=kt == 0, stop=kt == pnkt - 1)
                        stt["k"] += 1
                        n -= 1
                    if stt["k"] >= pnkt and not stt.get("fin"):
                        finalize(pb_, ph, pqc, pn)
                        stt["fin"] = True
                    return stt.get("fin", False)

                for b in range(B):
                    fillers = []
                    if b == 0:
                        fillers += p1_half_closures(2, 0)
                        fillers += p1_half_closures(2, 1)
                    elif b == 1:
                        fillers += p1_half_closures(3, 0)
                        if 3 in phases:
                            for s in range(8):
                                fillers += p3_stripe_closures(0, s)
                    elif b == 2:
                        fillers += p1_half_closures(3, 1)
                        if 3 in phases:
                            for s in range(8):
                                fillers += p3_stripe_closures(1, s)
                    else:
                        if 3 in phases:
                            for s in range(8):
                                fillers += p3_stripe_closures(2, s)
                    fi = 0
                    for h, qc in units:
                        nkt = 4 * qc + 4
                        es = esp.tile([128, 8, 512], BF, tag="es", name="es")
                        rhs = q_rhs(b, h, qc)
                        for kt in range(nkt):
                            psc = sps.tile([128, 512], F32, tag="psc",
                                           name="psc")
                            nc.tensor.matmul(psc[:], k_lhsT(b, h, kt), rhs,
                                             start=True, stop=True)
                            nc.scalar.activation(es[:, kt, :], psc[:],
                                                 AF.Exp, scale=0.125)
                            if kt >= 4 * qc:
                                nc.gpsimd.tensor_tensor(
                                    es[:, kt, :], es[:, kt, :],
                                    mask_sb[:, kt - 4 * qc, :], MULT)
                            if prev is not None:
                                emit_prev_num(prev, 2)
                            if fi < len(fillers) and kt % 2 == 1:
                                fillers[fi]()
                                fi += 1
                        if prev is not None:
                            while not emit_prev_num(prev, 4):
                                pass
                        prev = (b, h, qc, es, {})
                    while fi < len(fillers):
                        fillers[fi]()
                        fi += 1
                # drain last unit + P3(3)
                while not emit_prev_num(prev, 4):
                    pass
                if dbg:
                    for b in range(B):
                        sl = slice(b * T, (b + 1) * T)
                        nc.sync.dma_start(d_qt.ap()[:, sl], QT[b][:])
                        nc.sync.dma_start(d_kt.ap()[:, sl], KT[b][:])
                        nc.sync.dma_start(d_qt2.ap()[:, sl], QT2[b][:])
                        nc.sync.dma_start(d_kt2.ap()[:, sl], KT2[b][:])
                        n = 8 * NH * 65
                        nc.sync.dma_start(
                            d_vh.ap()[:, b * n:(b + 1) * n],
                            VH[b][:].rearrange("p a h o -> p (a h o)"))
                        nc.sync.dma_start(d_hot.ap()[:, sl], HOT[b][:])
                if 3 in phases:
                    for s in range(8):
                        for c in p3_stripe_closures(3, s):
                            c()

            if reps == 1:
                emit()
            else:
                with tc.For_i(0, reps, 1):
                    emit()
    nc.compile()
    return nc


def _rne_bf16(a):
    """float32 ndarray -> bfloat16 (round to nearest even), fast path."""
    import ml_dtypes
    v = np.ascontiguousarray(a, dtype=np.float32).view(np.uint32)
    r = ((v >> 16) & 1) + 0x7FFF
    return ((v + r) >> 16).astype(np.uint16).view(ml_dtypes.bfloat16)


def prep_in_maps(x, W_Q, W_K, W_V, W_O, FK0, PK0, FV0, PV0, FK1, PK1, FV1, PV1):
    x = np.asarray(x, dtype=np.float32)
    W_K_eff = np.array(W_K, dtype=np.float32, copy=True)
    W_V_eff = np.array(W_V, dtype=np.float32, copy=True)
    for tier, (FK, PK, FV, PV) in {0: (FK0, PK0, FV0, PV0),
                                   1: (FK1, PK1, FV1, PV1)}.items():
        FK = np.asarray(FK); PK = np.asarray(PK)
        FV = np.asarray(FV); PV = np.asarray(PV)
        lo = IN_OFF[tier + 1]
        for h in range(NHS[tier]):
            col = OUT_OFF[tier] + h * DK
            W_K_eff[lo:, col:col + DK] += FK[:, h * RANK:(h + 1) * RANK] @ PK[h]
            W_V_eff[lo:, col:col + DK] += FV[:, h * RANK:(h + 1) * RANK] @ PV[h]
    W_Q = np.asarray(W_Q, dtype=np.float32)
    W_O = np.asarray(W_O, dtype=np.float32)

    xT_bf = np.ascontiguousarray(_rne_bf16(x.reshape(BT, D)).T)

    k = np.arange(128)[:, None]
    q = np.arange(512)[None, :]
    msk = np.concatenate([(q >= 128 * i + k).astype(np.float32)
                          for i in range(4)] + [np.ones((128, 32), np.float32)],
                         axis=1)
    msk_bf = _rne_bf16(msk)
    cst = np.ones((1, 64), dtype=np.float32)

    in_maps = []
    for c in range(NCORES):
        lo = c * NH * DK
        hi = lo + NH * DK
        wqkc = np.concatenate([W_Q[:, lo:lo + 128], W_K_eff[:, lo:lo + 128],
                               W_Q[:, lo + 128:hi], W_K_eff[:, lo + 128:hi]],
                              axis=1)
        woc = np.zeros((256, D), dtype=np.float32)
        woc[0:128] = W_O[lo:lo + 128]
        woc[192:256] = W_O[lo + 128:hi]
        in_maps.append({
            "xT": xT_bf,
            "wqk": _rne_bf16(np.ascontiguousarray(wqkc)),
            "wv": _rne_bf16(np.ascontiguousarray(W_V_eff[:, lo:hi])),
            "wo": _rne_bf16(woc),
            "msk": msk_bf,
            "cst": cst,
        })
    return in_maps


_NC_CACHE = []
_EXEC_CACHE = {}


def get_nc():
    if not _NC_CACHE:
        _NC_CACHE.append(build_nc())
    return _NC_CACHE[0]


def _get_exec(nc, n_cores):
    key = id(nc)
    if key in _EXEC_CACHE:
        return _EXEC_CACHE[key]
    import jax
    from jax.sharding import Mesh, PartitionSpec, NamedSharding
    from jax.experimental.shard_map import shard_map

    bass2jax.install_neuronx_cc_hook()
    partition_name = (nc.partition_id_tensor.name
                      if nc.partition_id_tensor else None)
    in_names, out_names, out_avals, zero_outs = [], [], [], []
    for alloc in nc.m.functions[0].allocations:
        if not isinstance(alloc, mybir.MemoryLocationSet):
            continue
        name = alloc.memorylocations[0].name
        if alloc.kind == "ExternalInput":
            if name != partition_name:
                in_names.append(name)
        elif alloc.kind == "ExternalOutput":
            out_names.append(name)
            shape = tuple(alloc.tensor_shape)
            dtype = mybir.dt.np(alloc.dtype)
            out_avals.append(jax.core.ShapedArray(shape, dtype))
            zero_outs.append(np.zeros(shape, dtype))
    all_in_names = list(in_names) + list(out_names)
    if partition_name is not None:
        all_in_names.append(partition_name)

    def _body(*args):
        operands = list(args)
        if partition_name is not None:
            operands.append(bass2jax.partition_id_tensor())
        outs = bass2jax._bass_exec_p.bind(
            *operands,
            out_avals=tuple(out_avals),
            in_names=tuple(all_in_names),
            out_names=tuple(out_names),
            lowering_input_output_aliases=(),
            sim_require_finite=True,
            sim_require_nnan=True,
            nc=nc,
        )
        return tuple(outs)

    devices = jax.devices()[:n_cores]
    mesh = Mesh(np.asarray(devices), ("core",))
    spec = NamedSharding(mesh, PartitionSpec("core"))
    n_params = len(in_names)
    fn = jax.jit(shard_map(
        _body, mesh=mesh,
        in_specs=(PartitionSpec("core"),) * (n_params + len(out_names)),
        out_specs=(PartitionSpec("core"),) * len(out_names),
        check_rep=False), keep_unused=True)
    entry = dict(fn=fn, in_names=in_names, out_names=out_names,
                 out_avals=out_avals, zero_outs=zero_outs, spec=spec,
                 devices=devices, dev_zeros=None)
    _EXEC_CACHE[key] = entry
    return entry


def _put_sharded(arrs, devices, spec):
    """Put per-core host arrays on devices without a host-side concat."""
    import jax
    shape0 = arrs[0].shape
    global_shape = (len(arrs) * shape0[0],) + tuple(shape0[1:])
    shards = [jax.device_put(a, d) for a, d in zip(arrs, devices)]
    return jax.make_array_from_single_device_arrays(global_shape, spec, shards)


def kernel(**inputs):
    import jax
    nc = get_nc()
    ex = _get_exec(nc, NCORES)
    in_maps = prep_in_maps(**inputs)
    concat_in = [
        _put_sharded([in_maps[c][nm] for c in range(NCORES)],
                     ex["devices"], ex["spec"])
        for nm in ex["in_names"]
    ]
    if ex["dev_zeros"] is None:
        ex["dev_zeros"] = [
            _put_sharded([z] * NCORES, ex["devices"], ex["spec"])
            for z in ex["zero_outs"]
        ]
    outs = ex["fn"](*concat_in, *ex["dev_zeros"])
    jax.block_until_ready(outs)
    o = np.asarray(outs[0]).reshape(NCORES, BT, D)
    acc = o.astype(np.float32).sum(axis=0)
    return acc.reshape(B, T, D)


# revision 3
# speedup vs baseline: 1.1259x; 1.1259x over previous
"""Matryoshka attention Trainium2 kernel v2: 8-core SPMD, head-parallel,
bf16, batch-software-pipelined.

24 heads across 3 tiers -> 3 heads/core; feedback folded into dense
effective K/V projections on host. All operands bf16 (f32 PSUM
accumulation); rel err ~4e-3 vs 2e-2 tolerance.

Per core, per batch b (T=1024 tokens):
  P1(b): project Q^T,K^T (dk-major) and V-hat (token-major + ones col)
         for its 3 heads from a resident bf16 x^T column block.
  P2(b): causal attention, transposed scores S^T = K Q^T, exp on ACT
         (scores bounded, no max subtraction), causal mask via mult on
         Pool (gpsimd), denominator via ones column, normalization via
         K=1 broadcast matmul of the reciprocal row.
  P3(b): partial out stripes = headout^T @ W_O rows, bf16 out DMA;
         host sums the 8 partials.
Emission interleaves P1(b+2)/P1(b+3) chunks and P3(b-1) stripes between
P2(b) score steps so the PE queue never drains; num matmuls of unit j-1
are interleaved kt-by-kt with unit j's score matmuls.
"""

import sys

if "/opt/trn_rl_repo" not in sys.path:
    sys.path.insert(0, "/opt/trn_rl_repo")

import numpy as np

import concourse.bass as bass
import concourse.tile as tile
from concourse import bacc, mybir
from concourse import bass2jax

F32 = mybir.dt.float32
F32R = mybir.dt.float32r
BF = mybir.dt.bfloat16
AF = mybir.ActivationFunctionType
MULT = mybir.AluOpType.mult

B, T, D = 4, 1024, 2048
BT = B * T
DK = 64
NH = 3
NCORES = 8
IN_OFF = [0, 256, 1024, 2048]
OUT_OFF = [0, 256, 768, 1536]
NHS = [4, 8, 12]
RANK = 8
KD = D // 128  # 16 contraction chunks


def build_nc(dbg=False, reps=1, phases=(1, 2, 3)):
    nc = bacc.Bacc("TRN2", target_bir_lowering=False, debug=False)
    xT = nc.dram_tensor("xT", [D, BT], BF, kind="ExternalInput")
    wqk = nc.dram_tensor("wqk", [D, 384], BF, kind="ExternalInput")
    wv = nc.dram_tensor("wv", [D, 192], BF, kind="ExternalInput")
    wo = nc.dram_tensor("wo", [256, D], BF, kind="ExternalInput")
    # msk: cols 0:2048 = 4 causal diagonal masks, cols 2048:2080 = ones
    msk = nc.dram_tensor("msk", [128, 2080], BF, kind="ExternalInput")
    cst = nc.dram_tensor("cst", [1, 64], F32, kind="ExternalInput")
    out = nc.dram_tensor("out", [BT, D], BF, kind="ExternalOutput")
    if dbg:
        d_qt = nc.dram_tensor("d_qt", [128, BT], BF, kind="ExternalOutput")
        d_kt = nc.dram_tensor("d_kt", [128, BT], BF, kind="ExternalOutput")
        d_qt2 = nc.dram_tensor("d_qt2", [64, BT], BF, kind="ExternalOutput")
        d_kt2 = nc.dram_tensor("d_kt2", [128, BT], BF, kind="ExternalOutput")
        d_vh = nc.dram_tensor("d_vh", [128, B * 8 * NH * 65], BF,
                              kind="ExternalOutput")
        d_hot = nc.dram_tensor("d_hot", [128, BT], BF, kind="ExternalOutput")

    with tile.TileContext(nc) as tc:
        with tc.tile_pool(name="pers", bufs=1) as pers, \
             tc.tile_pool(name="xp", bufs=2) as xp, \
             tc.tile_pool(name="esp", bufs=4) as esp, \
             tc.tile_pool(name="recp", bufs=2) as recp, \
             tc.tile_pool(name="osp", bufs=3) as osp, \
             tc.tile_pool(name="qkps", bufs=1, space="PSUM") as qkps, \
             tc.tile_pool(name="sps", bufs=3, space="PSUM") as sps, \
             tc.tile_pool(name="nps", bufs=2, space="PSUM") as nps, \
             tc.tile_pool(name="bp3", bufs=2, space="PSUM") as bp3:
            wqk_sb = pers.tile([128, KD, 384], BF)
            wv_sb = pers.tile([128, KD, 192], BF)
            wo_sb = pers.tile([128, 2, D], BF)
            mask_sb = pers.tile([128, 4, 512], BF)
            ones_sb = pers.tile([1, 64], F32R)
            QT = [pers.tile([128, T], BF, name=f"qt{b}") for b in range(B)]
            KT = [pers.tile([128, T], BF, name=f"kt{b}") for b in range(B)]
            QT2 = [pers.tile([64, T], BF, name=f"qt2_{b}") for b in range(B)]
            # KT2 rows 0:64 = K^T of head 2; rows 64:128 reused as hoT of head 2
            KT2 = [pers.tile([128, T], BF, name=f"kt2_{b}") for b in range(B)]
            VH = [pers.tile([128, 8, NH, 65], BF, name=f"vh{b}") for b in range(B)]
            HOT = [pers.tile([128, T], BF, name=f"hot{b}") for b in range(B)]

            # prelude: first x block interleaved with chunked weight loads so
            # the first matmuls start ~3us in instead of ~17us
            xs00 = pers.tile([128, KD, 512], BF, name="xs00")
            for kc in range(4):
                nc.sync.dma_start(
                    xs00[:, kc * 4:(kc + 1) * 4, :],
                    xT.ap()[kc * 512:(kc + 1) * 512, 0:512].rearrange(
                        "(k p) n -> p k n", p=128))
                nc.sync.dma_start(
                    wqk_sb[:, kc * 4:(kc + 1) * 4, :],
                    wqk.ap()[kc * 512:(kc + 1) * 512, :].rearrange(
                        "(k p) n -> p k n", p=128))
                nc.sync.dma_start(
                    wv_sb[:, kc * 4:(kc + 1) * 4, :],
                    wv.ap()[kc * 512:(kc + 1) * 512, :].rearrange(
                        "(k p) n -> p k n", p=128))
            nc.sync.dma_start(wo_sb[:], wo.ap().rearrange(
                "(k p) n -> p k n", p=128))
            nc.sync.dma_start(mask_sb[:], msk.ap()[:, 0:2048].rearrange(
                "p (i n) -> p i n", i=4))
            nc.sync.dma_start(ones_sb[:], cst.ap().bitcast(F32R))

            def emit():
                for b in range(B):
                    nc.sync.dma_start(
                        VH[b][:, :, :, 64:65],
                        msk.ap()[:, 2048:2072].rearrange(
                            "p (k h o) -> p k h o", k=8, o=1))

                # ---------- P1 chunk closures ----------
                def p1_half_closures(b, half):
                    col = b * T + half * 512
                    st = {}

                    def a_dma():
                        if b == 0 and half == 0:
                            st["xs"] = xs00
                            return
                        xs = xp.tile([128, KD, 512], BF, tag="xs", name="xs")
                        for kc in range(4):
                            nc.sync.dma_start(
                                xs[:, kc * 4:(kc + 1) * 4, :],
                                xT.ap()[kc * 512:(kc + 1) * 512,
                                        col:col + 512].rearrange(
                                    "(k p) n -> p k n", p=128))
                        st["xs"] = xs

                    def a_qk(mt):
                        def f():
                            xs = st["xs"]
                            ps = qkps.tile([128, 512], F32, tag="qk", name="qk")
                            for kd in range(KD):
                                nc.tensor.matmul(
                                    ps[:], wqk_sb[:, kd, mt * 128:(mt + 1) * 128],
                                    xs[:, kd, :], start=kd == 0, stop=kd == KD - 1)
                            dst = half * 512
                            if mt == 0:
                                nc.vector.tensor_copy(
                                    QT[b][:, dst:dst + 512], ps[:])
                            elif mt == 1:
                                nc.scalar.copy(KT[b][:, dst:dst + 512], ps[:])
                            else:
                                nc.vector.tensor_copy(
                                    QT2[b][0:64, dst:dst + 512], ps[0:64, :])
                                nc.scalar.copy(
                                    KT2[b][0:64, dst:dst + 512], ps[64:128, :])
                        return f

                    def a_v(pr):
                        def f():
                            xs = st["xs"]
                            for s2 in range(2):
                                sub = pr * 2 + s2
                                pv = nps.tile([128, 192], F32, tag="pn",
                                              name="pv")
                                for kd in range(KD):
                                    nc.tensor.matmul(
                                        pv[:],
                                        xs[:, kd, sub * 128:(sub + 1) * 128],
                                        wv_sb[:, kd, :],
                                        start=kd == 0, stop=kd == KD - 1)
                                so = half * 4 + sub
                                nc.vector.tensor_copy(
                                    VH[b][:, so, :, 0:64],
                                    pv[:].rearrange("p (h d) -> p h d", h=NH))
                        return f

                    return [a_dma, a_qk(0), a_v(0), a_qk(1), a_v(1), a_qk(2)]

                # ---------- P3 stripe closures ----------
                def p3_stripe_closures(b, s):
                    ms = slice(b * T + s * 128, b * T + (s + 1) * 128)
                    ls = slice(s * 128, (s + 1) * 128)
                    st = {}

                    def a_pair(pair):
                        def f():
                            if "osb" not in st:
                                st["osb"] = osp.tile([128, D], BF, tag="osb",
                                                     name="osb")
                            osb = st["osb"]
                            tiles = []
                            for j in range(2):
                                nt = pair * 2 + j
                                po = bp3.tile([128, 512], F32, tag="bp3",
                                              name="po")
                                nc.tensor.matmul(
                                    po[:], HOT[b][:, ls],
                                    wo_sb[:, 0, nt * 512:(nt + 1) * 512],
                                    start=True, stop=False)
                                tiles.append((nt, po))
                            for j, (nt, po) in enumerate(tiles):
                                nc.tensor.matmul(
                                    po[:], KT2[b][64:128, ls],
                                    wo_sb[64:128, 1, nt * 512:(nt + 1) * 512],
                                    start=False, stop=True)
                            for j, (nt, po) in enumerate(tiles):
                                ns = slice(nt * 512, (nt + 1) * 512)
                                if j == 0:
                                    nc.vector.tensor_copy(osb[:, ns], po[:])
                                else:
                                    nc.scalar.copy(osb[:, ns], po[:])
                            if pair == 1:
                                nc.sync.dma_start(out.ap()[ms, :], osb[:])
                        return f

                    return [a_pair(0), a_pair(1)]

                # ---------- P2 helpers ----------
                def k_lhsT(b, h, kt):
                    sl = slice(kt * 128, (kt + 1) * 128)
                    if h == 0:
                        return KT[b][0:64, sl]
                    if h == 1:
                        return KT[b][64:128, sl]
                    return KT2[b][0:64, sl]

                def q_rhs(b, h, qc):
                    sl = slice(qc * 512, (qc + 1) * 512)
                    if h == 0:
                        return QT[b][0:64, sl]
                    if h == 1:
                        return QT[b][64:128, sl]
                    return QT2[b][0:64, sl]

                def finalize(b, h, qc, pn):
                    qoff = qc * 512
                    rec = recp.tile([1, 512], F32R, tag="rec", name="rec")
                    with nc.allow_low_precision(
                            reason="softmax denominator reciprocal"):
                        nc.vector.reciprocal(rec[:], pn[64:65, :])
                    pb = bp3.tile([128, 512], F32, tag="bp3", name="pb")
                    nc.tensor.matmul(pb[0:64, :], ones_sb[:], rec[:],
                                     start=True, stop=True)
                    bc = recp.tile([64, 512], F32R, tag="bc", name="bc")
                    nc.vector.tensor_copy(bc[:], pb[0:64, :])
                    if h == 0:
                        dest = HOT[b][0:64, qoff:qoff + 512]
                    elif h == 1:
                        dest = HOT[b][64:128, qoff:qoff + 512]
                    else:
                        dest = KT2[b][64:128, qoff:qoff + 512]
                    nc.vector.tensor_tensor(dest, pn[0:64, :], bc[:], MULT)

                # ---------- pipeline ----------
                if 2 not in phases:
                    for b in range(B):
                        for c in p1_half_closures(b, 0):
                            c()
                        for c in p1_half_closures(b, 1):
                            c()
                    return

                # prologue: P1 of batches 0 and 1
                for b in (0, 1):
                    for half in (0, 1):
                        for c in p1_half_closures(b, half):
                            c()

                from collections import deque
                pending = deque()  # dicts: b, h, qc, es, pn, k

                def drain_pending(n):
                    """Emit up to n num matmuls from the pending queue."""
                    while n > 0 and pending:
                        p = pending[0]
                        pnkt = 4 * p["qc"] + 4
                        if "pn" not in p:
                            p["pn"] = nps.tile([128, 512], F32, tag="pn",
                                               name="pn")
                        while n > 0 and p["k"] < pnkt:
                            kt = p["k"]
                            nc.tensor.matmul(
                                p["pn"][0:65, :], VH[p["b"]][:, kt, p["h"], :],
                                p["es"][:, kt, :],
                                start=kt == 0, stop=kt == pnkt - 1)
                            p["k"] += 1
                            n -= 1
                        if p["k"] >= pnkt:
                            finalize(p["b"], p["h"], p["qc"], p["pn"])
                            pending.popleft()

                for b in range(B):
                    fillers = []
                    if b == 0:
                        fillers += p1_half_closures(2, 0)
                        fillers += p1_half_closures(2, 1)
                    elif b == 1:
                        fillers += p1_half_closures(3, 0)
                        if 3 in phases:
                            for s in range(8):
                                fillers += p3_stripe_closures(0, s)
                    elif b == 2:
                        fillers += p1_half_closures(3, 1)
                        if 3 in phases:
                            for s in range(8):
                                fillers += p3_stripe_closures(1, s)
                    else:
                        if 3 in phases:
                            for s in range(8):
                                fillers += p3_stripe_closures(2, s)
                    fi = 0
                    # super-units: heads 0+1 paired (adjacent score matmuls in
                    # PE row groups 0 and 64 run concurrently), head 2 alone
                    for su, qc in [(0, 0), (0, 1), (2, 0), (2, 1)]:
                        nkt = 4 * qc + 4
                        heads = (0, 1) if su == 0 else (2,)
                        ess = {h: esp.tile([128, 8, 512], BF, tag="es",
                                           name="es") for h in heads}
                        for kt in range(nkt):
                            for h in heads:
                                psc = sps.tile([128, 512], F32, tag="psc",
                                               name="psc")
                                nc.tensor.matmul(psc[:], k_lhsT(b, h, kt),
                                                 q_rhs(b, h, qc),
                                                 start=True, stop=True)
                                nc.scalar.activation(ess[h][:, kt, :], psc[:],
                                                     AF.Exp, scale=0.125)
                                if kt >= 4 * qc:
                                    nc.gpsimd.tensor_tensor(
                                        ess[h][:, kt, :], ess[h][:, kt, :],
                                        mask_sb[:, kt - 4 * qc, :], MULT)
                            drain_pending(3 if su == 0 else 2)
                            if fi < len(fillers) and kt % 2 == 1:
                                fillers[fi]()
                                fi += 1
                        for h in heads:
                            pending.append(dict(b=b, h=h, qc=qc, es=ess[h],
                                                k=0))
                    while fi < len(fillers):
                        fillers[fi]()
                        fi += 1
                # drain remaining nums + P3(3)
                drain_pending(10 ** 9)
                if dbg:
                    for b in range(B):
                        sl = slice(b * T, (b + 1) * T)
                        nc.sync.dma_start(d_qt.ap()[:, sl], QT[b][:])
                        nc.sync.dma_start(d_kt.ap()[:, sl], KT[b][:])
                        nc.sync.dma_start(d_qt2.ap()[:, sl], QT2[b][:])
                        nc.sync.dma_start(d_kt2.ap()[:, sl], KT2[b][:])
                        n = 8 * NH * 65
                        nc.sync.dma_start(
                            d_vh.ap()[:, b * n:(b + 1) * n],
                            VH[b][:].rearrange("p a h o -> p (a h o)"))
                        nc.sync.dma_start(d_hot.ap()[:, sl], HOT[b][:])
                if 3 in phases:
                    for s in range(8):
                        for c in p3_stripe_closures(3, s):
                            c()

            if reps == 1:
                emit()
            else:
                with tc.For_i(0, reps, 1):
                    emit()
    nc.compile()
    return nc


def _rne_bf16(a):
    """float32 ndarray -> bfloat16 (round to nearest even), fast path."""
    import ml_dtypes
    v = np.ascontiguousarray(a, dtype=np.float32).view(np.uint32)
    r = ((v >> 16) & 1) + 0x7FFF
    return ((v + r) >> 16).astype(np.uint16).view(ml_dtypes.bfloat16)


def prep_in_maps(x, W_Q, W_K, W_V, W_O, FK0, PK0, FV0, PV0, FK1, PK1, FV1, PV1):
    x = np.asarray(x, dtype=np.float32)
    W_K_eff = np.array(W_K, dtype=np.float32, copy=True)
    W_V_eff = np.array(W_V, dtype=np.float32, copy=True)
    for tier, (FK, PK, FV, PV) in {0: (FK0, PK0, FV0, PV0),
                                   1: (FK1, PK1, FV1, PV1)}.items():
        FK = np.asarray(FK); PK = np.asarray(PK)
        FV = np.asarray(FV); PV = np.asarray(PV)
        lo = IN_OFF[tier + 1]
        for h in range(NHS[tier]):
            col = OUT_OFF[tier] + h * DK
            W_K_eff[lo:, col:col + DK] += FK[:, h * RANK:(h + 1) * RANK] @ PK[h]
            W_V_eff[lo:, col:col + DK] += FV[:, h * RANK:(h + 1) * RANK] @ PV[h]
    W_Q = np.asarray(W_Q, dtype=np.float32)
    W_O = np.asarray(W_O, dtype=np.float32)

    xT_bf = np.ascontiguousarray(_rne_bf16(x.reshape(BT, D)).T)

    k = np.arange(128)[:, None]
    q = np.arange(512)[None, :]
    msk = np.concatenate([(q >= 128 * i + k).astype(np.float32)
                          for i in range(4)] + [np.ones((128, 32), np.float32)],
                         axis=1)
    msk_bf = _rne_bf16(msk)
    cst = np.ones((1, 64), dtype=np.float32)

    in_maps = []
    for c in range(NCORES):
        lo = c * NH * DK
        hi = lo + NH * DK
        wqkc = np.concatenate([W_Q[:, lo:lo + 128], W_K_eff[:, lo:lo + 128],
                               W_Q[:, lo + 128:hi], W_K_eff[:, lo + 128:hi]],
                              axis=1)
        woc = np.zeros((256, D), dtype=np.float32)
        woc[0:128] = W_O[lo:lo + 128]
        woc[192:256] = W_O[lo + 128:hi]
        in_maps.append({
            "xT": xT_bf,
            "wqk": _rne_bf16(np.ascontiguousarray(wqkc)),
            "wv": _rne_bf16(np.ascontiguousarray(W_V_eff[:, lo:hi])),
            "wo": _rne_bf16(woc),
            "msk": msk_bf,
            "cst": cst,
        })
    return in_maps


_NC_CACHE = []
_EXEC_CACHE = {}


def get_nc():
    if not _NC_CACHE:
        _NC_CACHE.append(build_nc())
    return _NC_CACHE[0]


def _get_exec(nc, n_cores):
    key = id(nc)
    if key in _EXEC_CACHE:
        return _EXEC_CACHE[key]
    import jax
    from jax.sharding import Mesh, PartitionSpec, NamedSharding
    from jax.experimental.shard_map import shard_map

    bass2jax.install_neuronx_cc_hook()
    partition_name = (nc.partition_id_tensor.name
                      if nc.partition_id_tensor else None)
    in_names, out_names, out_avals, zero_outs = [], [], [], []
    for alloc in nc.m.functions[0].allocations:
        if not isinstance(alloc, mybir.MemoryLocationSet):
            continue
        name = alloc.memorylocations[0].name
        if alloc.kind == "ExternalInput":
            if name != partition_name:
                in_names.append(name)
        elif alloc.kind == "ExternalOutput":
            out_names.append(name)
            shape = tuple(alloc.tensor_shape)
            dtype = mybir.dt.np(alloc.dtype)
            out_avals.append(jax.core.ShapedArray(shape, dtype))
            zero_outs.append(np.zeros(shape, dtype))
    all_in_names = list(in_names) + list(out_names)
    if partition_name is not None:
        all_in_names.append(partition_name)

    def _body(*args):
        operands = list(args)
        if partition_name is not None:
            operands.append(bass2jax.partition_id_tensor())
        outs = bass2jax._bass_exec_p.bind(
            *operands,
            out_avals=tuple(out_avals),
            in_names=tuple(all_in_names),
            out_names=tuple(out_names),
            lowering_input_output_aliases=(),
            sim_require_finite=True,
            sim_require_nnan=True,
            nc=nc,
        )
        return tuple(outs)

    devices = jax.devices()[:n_cores]
    mesh = Mesh(np.asarray(devices), ("core",))
    spec = NamedSharding(mesh, PartitionSpec("core"))
    n_params = len(in_names)
    fn = jax.jit(shard_map(
        _body, mesh=mesh,
        in_specs=(PartitionSpec("core"),) * (n_params + len(out_names)),
        out_specs=(PartitionSpec("core"),) * len(out_names),
        check_rep=False), keep_unused=True)
    entry = dict(fn=fn, in_names=in_names, out_names=out_names,
                 out_avals=out_avals, zero_outs=zero_outs, spec=spec,
                 devices=devices, dev_zeros=None)
    _EXEC_CACHE[key] = entry
    return entry


def _put_sharded(arrs, devices, spec):
    """Put per-core host arrays on devices without a host-side concat."""
    import jax
    shape0 = arrs[0].shape
    global_shape = (len(arrs) * shape0[0],) + tuple(shape0[1:])
    shards = [jax.device_put(a, d) for a, d in zip(arrs, devices)]
    return jax.make_array_from_single_device_arrays(global_shape, spec, shards)


def kernel(**inputs):
    import jax
    nc = get_nc()
    ex = _get_exec(nc, NCORES)
    in_maps = prep_in_maps(**inputs)
    concat_in = [
        _put_sharded([in_maps[c][nm] for c in range(NCORES)],
                     ex["devices"], ex["spec"])
        for nm in ex["in_names"]
    ]
    if ex["dev_zeros"] is None:
        ex["dev_zeros"] = [
            _put_sharded([z] * NCORES, ex["devices"], ex["spec"])
            for z in ex["zero_outs"]
        ]
    outs = ex["fn"](*concat_in, *ex["dev_zeros"])
    jax.block_until_ready(outs)
    o = np.asarray(outs[0]).reshape(NCORES, BT, D)
    acc = o.astype(np.float32).sum(axis=0)
    return acc.reshape(B, T, D)


# revision 4
# speedup vs baseline: 1.4029x; 1.2461x over previous
"""Matryoshka attention Trainium2 kernel v2: 8-core SPMD, head-parallel,
bf16, batch-software-pipelined.

24 heads across 3 tiers -> 3 heads/core; feedback folded into dense
effective K/V projections on host. All operands bf16 (f32 PSUM
accumulation); rel err ~4e-3 vs 2e-2 tolerance.

Per core, per batch b (T=1024 tokens):
  P1(b): project Q^T,K^T (dk-major) and V-hat (token-major + ones col)
         for its 3 heads from a resident bf16 x^T column block.
  P2(b): causal attention, transposed scores S^T = K Q^T, exp on ACT
         (scores bounded, no max subtraction), causal mask via mult on
         Pool (gpsimd), denominator via ones column, normalization via
         K=1 broadcast matmul of the reciprocal row.
  P3(b): partial out stripes = headout^T @ W_O rows, bf16 out DMA;
         host sums the 8 partials.
Emission interleaves P1(b+2)/P1(b+3) chunks and P3(b-1) stripes between
P2(b) score steps so the PE queue never drains; num matmuls of unit j-1
are interleaved kt-by-kt with unit j's score matmuls.
"""

import sys

if "/opt/trn_rl_repo" not in sys.path:
    sys.path.insert(0, "/opt/trn_rl_repo")

import numpy as np

import concourse.bass as bass
import concourse.tile as tile
from concourse import bacc, mybir
from concourse import bass2jax

F32 = mybir.dt.float32
F32R = mybir.dt.float32r
BF = mybir.dt.bfloat16
AF = mybir.ActivationFunctionType
MULT = mybir.AluOpType.mult

B, T, D = 4, 1024, 2048
BT = B * T
DK = 64
NH = 3
NCORES = 8
IN_OFF = [0, 256, 1024, 2048]
OUT_OFF = [0, 256, 768, 1536]
NHS = [4, 8, 12]
RANK = 8
KD = D // 128  # 16 contraction chunks


def build_nc(dbg=False, reps=1, phases=(1, 2, 3)):
    nc = bacc.Bacc("TRN2", target_bir_lowering=False, debug=False)
    xT = nc.dram_tensor("xT", [D, BT], BF, kind="ExternalInput")
    wqk = nc.dram_tensor("wqk", [D, 384], BF, kind="ExternalInput")
    wv = nc.dram_tensor("wv", [D, 192], BF, kind="ExternalInput")
    wo = nc.dram_tensor("wo", [256, D], BF, kind="ExternalInput")
    # msk: cols 0:2048 = 4 causal diagonal masks, cols 2048:2080 = ones
    msk = nc.dram_tensor("msk", [128, 2080], BF, kind="ExternalInput")
    cst = nc.dram_tensor("cst", [1, 64], F32, kind="ExternalInput")
    out = nc.dram_tensor("out", [BT, D], BF, kind="ExternalOutput")
    if dbg:
        d_qt = nc.dram_tensor("d_qt", [128, BT], BF, kind="ExternalOutput")
        d_kt = nc.dram_tensor("d_kt", [128, BT], BF, kind="ExternalOutput")
        d_qt2 = nc.dram_tensor("d_qt2", [64, BT], BF, kind="ExternalOutput")
        d_kt2 = nc.dram_tensor("d_kt2", [128, BT], BF, kind="ExternalOutput")
        d_vh = nc.dram_tensor("d_vh", [128, B * 8 * NH * 65], BF,
                              kind="ExternalOutput")
        d_hot = nc.dram_tensor("d_hot", [128, BT], BF, kind="ExternalOutput")

    with tile.TileContext(nc) as tc:
        with tc.tile_pool(name="pers", bufs=1) as pers, \
             tc.tile_pool(name="xp", bufs=2) as xp, \
             tc.tile_pool(name="esp", bufs=4) as esp, \
             tc.tile_pool(name="recp", bufs=2) as recp, \
             tc.tile_pool(name="osp", bufs=4) as osp, \
             tc.tile_pool(name="qkps", bufs=1, space="PSUM") as qkps, \
             tc.tile_pool(name="sps", bufs=3, space="PSUM") as sps, \
             tc.tile_pool(name="nps", bufs=2, space="PSUM") as nps, \
             tc.tile_pool(name="bp3", bufs=2, space="PSUM") as bp3:
            wqk_sb = pers.tile([128, KD, 384], BF)
            wv_sb = pers.tile([128, KD, 192], BF)
            wo_sb = pers.tile([128, 2, D], BF)
            mask_sb = pers.tile([128, 4, 512], BF)
            ones_sb = pers.tile([1, 64], F32R)
            QT = [pers.tile([128, T], BF, name=f"qt{b}") for b in range(B)]
            KT = [pers.tile([128, T], BF, name=f"kt{b}") for b in range(B)]
            QT2 = [pers.tile([64, T], BF, name=f"qt2_{b}") for b in range(B)]
            # KT2 rows 0:64 = K^T of head 2; rows 64:128 reused as hoT of head 2
            KT2 = [pers.tile([128, T], BF, name=f"kt2_{b}") for b in range(B)]
            VH = [pers.tile([128, 8, NH, 65], BF, name=f"vh{b}") for b in range(B)]
            # head-2 output dup at partitions 0:64 for P3 row-group pairing
            HOT2 = [pers.tile([64, T], BF, name=f"hot2_{b}") for b in range(B)]
            HOT = [pers.tile([128, T], BF, name=f"hot{b}") for b in range(B)]

            # prelude: first x block interleaved with chunked weight loads so
            # the first matmuls start ~3us in instead of ~17us
            xs00 = pers.tile([128, KD, 512], BF, name="xs00")
            for kc in range(4):
                nc.sync.dma_start(
                    xs00[:, kc * 4:(kc + 1) * 4, :],
                    xT.ap()[kc * 512:(kc + 1) * 512, 0:512].rearrange(
                        "(k p) n -> p k n", p=128))
                nc.sync.dma_start(
                    wqk_sb[:, kc * 4:(kc + 1) * 4, :],
                    wqk.ap()[kc * 512:(kc + 1) * 512, :].rearrange(
                        "(k p) n -> p k n", p=128))
                nc.sync.dma_start(
                    wv_sb[:, kc * 4:(kc + 1) * 4, :],
                    wv.ap()[kc * 512:(kc + 1) * 512, :].rearrange(
                        "(k p) n -> p k n", p=128))
            nc.sync.dma_start(wo_sb[:], wo.ap().rearrange(
                "(k p) n -> p k n", p=128))
            nc.sync.dma_start(mask_sb[:], msk.ap()[:, 0:2048].rearrange(
                "p (i n) -> p i n", i=4))
            nc.sync.dma_start(ones_sb[:], cst.ap().bitcast(F32R))

            def emit():
                for b in range(B):
                    nc.sync.dma_start(
                        VH[b][:, :, :, 64:65],
                        msk.ap()[:, 2048:2072].rearrange(
                            "p (k h o) -> p k h o", k=8, o=1))

                # ---------- P1 chunk closures ----------
                def p1_half_closures(b, half):
                    col = b * T + half * 512
                    st = {}

                    def a_dma():
                        if b == 0 and half == 0:
                            st["xs"] = xs00
                            return
                        xs = xp.tile([128, KD, 512], BF, tag="xs", name="xs")
                        for kc in range(4):
                            nc.sync.dma_start(
                                xs[:, kc * 4:(kc + 1) * 4, :],
                                xT.ap()[kc * 512:(kc + 1) * 512,
                                        col:col + 512].rearrange(
                                    "(k p) n -> p k n", p=128))
                        st["xs"] = xs

                    def a_qk(mt):
                        def f():
                            xs = st["xs"]
                            ps = qkps.tile([128, 512], F32, tag="qk", name="qk")
                            for kd in range(KD):
                                nc.tensor.matmul(
                                    ps[:], wqk_sb[:, kd, mt * 128:(mt + 1) * 128],
                                    xs[:, kd, :], start=kd == 0, stop=kd == KD - 1)
                            dst = half * 512
                            if mt == 0:
                                nc.vector.tensor_copy(
                                    QT[b][:, dst:dst + 512], ps[:])
                            elif mt == 1:
                                nc.scalar.copy(KT[b][:, dst:dst + 512], ps[:])
                            else:
                                nc.vector.tensor_copy(
                                    QT2[b][0:64, dst:dst + 512], ps[0:64, :])
                                nc.scalar.copy(
                                    KT2[b][0:64, dst:dst + 512], ps[64:128, :])
                        return f

                    def a_v(pr):
                        def f():
                            xs = st["xs"]
                            for s2 in range(2):
                                sub = pr * 2 + s2
                                pv = nps.tile([128, 192], F32, tag="pn",
                                              name="pv")
                                for kd in range(KD):
                                    nc.tensor.matmul(
                                        pv[:],
                                        xs[:, kd, sub * 128:(sub + 1) * 128],
                                        wv_sb[:, kd, :],
                                        start=kd == 0, stop=kd == KD - 1)
                                so = half * 4 + sub
                                nc.vector.tensor_copy(
                                    VH[b][:, so, :, 0:64],
                                    pv[:].rearrange("p (h d) -> p h d", h=NH))
                        return f

                    return [a_dma, a_qk(0), a_v(0), a_qk(1), a_v(1), a_qk(2)]

                # ---------- P3 stripe-pair closures ----------
                # Two adjacent stripes per group: their head-2 K=64 matmuls
                # use PE row groups 64:128 (from KT2) and 0:64 (dup copy in
                # QT2) so they run concurrently.
                def p3_pair_closures(b, sp):
                    s0, s1 = 2 * sp, 2 * sp + 1
                    ls0 = slice(s0 * 128, (s0 + 1) * 128)
                    ls1 = slice(s1 * 128, (s1 + 1) * 128)
                    st = {}

                    def a_nt(nt):
                        def f():
                            if "o0" not in st:
                                st["o0"] = osp.tile([128, D], BF, tag="osb",
                                                    name="osb")
                                st["o1"] = osp.tile([128, D], BF, tag="osb",
                                                    name="osb")
                            ns = slice(nt * 512, (nt + 1) * 512)
                            po_a = bp3.tile([128, 512], F32, tag="bp3",
                                            name="po")
                            po_b = bp3.tile([128, 512], F32, tag="bp3",
                                            name="po")
                            nc.tensor.matmul(po_a[:], HOT[b][:, ls0],
                                             wo_sb[:, 0, ns],
                                             start=True, stop=False)
                            nc.tensor.matmul(po_b[:], HOT[b][:, ls1],
                                             wo_sb[:, 0, ns],
                                             start=True, stop=False)
                            nc.tensor.matmul(po_a[:], KT2[b][64:128, ls0],
                                             wo_sb[64:128, 1, ns],
                                             start=False, stop=True)
                            nc.tensor.matmul(po_b[:], HOT2[b][0:64, ls1],
                                             wo_sb[0:64, 1, ns],
                                             start=False, stop=True)
                            nc.vector.tensor_copy(st["o0"][:, ns], po_a[:])
                            nc.scalar.copy(st["o1"][:, ns], po_b[:])
                            if nt == 3:
                                nc.sync.dma_start(
                                    out.ap()[b * T + s0 * 128:
                                             b * T + (s0 + 1) * 128, :],
                                    st["o0"][:])
                                nc.sync.dma_start(
                                    out.ap()[b * T + s1 * 128:
                                             b * T + (s1 + 1) * 128, :],
                                    st["o1"][:])
                        return f

                    return [a_nt(0), a_nt(1), a_nt(2), a_nt(3)]

                # ---------- P2 helpers ----------
                def k_lhsT(b, h, kt):
                    sl = slice(kt * 128, (kt + 1) * 128)
                    if h == 0:
                        return KT[b][0:64, sl]
                    if h == 1:
                        return KT[b][64:128, sl]
                    return KT2[b][0:64, sl]

                def q_rhs(b, h, qc):
                    sl = slice(qc * 512, (qc + 1) * 512)
                    if h == 0:
                        return QT[b][0:64, sl]
                    if h == 1:
                        return QT[b][64:128, sl]
                    return QT2[b][0:64, sl]

                def finalize(b, h, qc, pn):
                    qoff = qc * 512
                    rec = recp.tile([1, 512], F32R, tag="rec", name="rec")
                    with nc.allow_low_precision(
                            reason="softmax denominator reciprocal"):
                        nc.vector.reciprocal(rec[:], pn[64:65, :])
                    pb = bp3.tile([128, 512], F32, tag="bp3", name="pb")
                    nc.tensor.matmul(pb[0:64, :], ones_sb[:], rec[:],
                                     start=True, stop=True)
                    bc = recp.tile([64, 512], F32R, tag="bc", name="bc")
                    nc.vector.tensor_copy(bc[:], pb[0:64, :])
                    if h == 0:
                        dest = HOT[b][0:64, qoff:qoff + 512]
                    elif h == 1:
                        dest = HOT[b][64:128, qoff:qoff + 512]
                    else:
                        dest = KT2[b][64:128, qoff:qoff + 512]
                    nc.vector.tensor_tensor(dest, pn[0:64, :], bc[:], MULT)
                    if h == 2:
                        # dup copy at partitions 0:64 for P3's odd-stripe
                        # row-group pairing
                        nc.scalar.copy(HOT2[b][0:64, qoff:qoff + 512],
                                       KT2[b][64:128, qoff:qoff + 512])

                # ---------- pipeline ----------
                if 2 not in phases:
                    for b in range(B):
                        for c in p1_half_closures(b, 0):
                            c()
                        for c in p1_half_closures(b, 1):
                            c()
                    return

                # prologue: P1 of batch 0 only; P1(1)/P1(2) stream in as
                # batch-0 fillers so P2 starts sooner
                for half in (0, 1):
                    for c in p1_half_closures(0, half):
                        c()

                from collections import deque
                pending = deque()  # dicts: b, h, qc, es, pn, k

                def drain_pending(n):
                    """Emit up to n num matmuls from the pending queue."""
                    while n > 0 and pending:
                        p = pending[0]
                        pnkt = 4 * p["qc"] + 4
                        if "pn" not in p:
                            p["pn"] = nps.tile([128, 512], F32, tag="pn",
                                               name="pn")
                        while n > 0 and p["k"] < pnkt:
                            kt = p["k"]
                            nc.tensor.matmul(
                                p["pn"][0:65, :], VH[p["b"]][:, kt, p["h"], :],
                                p["es"][:, kt, :],
                                start=kt == 0, stop=kt == pnkt - 1)
                            p["k"] += 1
                            n -= 1
                        if p["k"] >= pnkt:
                            finalize(p["b"], p["h"], p["qc"], p["pn"])
                            pending.popleft()

                for b in range(B):
                    fillers = []
                    if b == 0:
                        fillers += p1_half_closures(1, 0)
                        fillers += p1_half_closures(1, 1)
                        fillers += p1_half_closures(2, 0)
                        fillers += p1_half_closures(2, 1)
                    elif b == 1:
                        fillers += p1_half_closures(3, 0)
                        if 3 in phases:
                            for sp in range(4):
                                fillers += p3_pair_closures(0, sp)
                    elif b == 2:
                        fillers += p1_half_closures(3, 1)
                        if 3 in phases:
                            for sp in range(4):
                                fillers += p3_pair_closures(1, sp)
                    else:
                        if 3 in phases:
                            for sp in range(4):
                                fillers += p3_pair_closures(2, sp)
                    fi = 0
                    # super-units: heads 0+1 paired (adjacent score matmuls in
                    # PE row groups 0 and 64 run concurrently), head 2 alone
                    for su, qc in [(0, 0), (0, 1), (2, 0), (2, 1)]:
                        nkt = 4 * qc + 4
                        heads = (0, 1) if su == 0 else (2,)
                        ess = {h: esp.tile([128, 8, 512], BF, tag="es",
                                           name="es") for h in heads}
                        for kt in range(nkt):
                            for h in heads:
                                psc = sps.tile([128, 512], F32, tag="psc",
                                               name="psc")
                                nc.tensor.matmul(psc[:], k_lhsT(b, h, kt),
                                                 q_rhs(b, h, qc),
                                                 start=True, stop=True)
                                nc.scalar.activation(ess[h][:, kt, :], psc[:],
                                                     AF.Exp, scale=0.125)
                                if kt >= 4 * qc:
                                    nc.gpsimd.tensor_tensor(
                                        ess[h][:, kt, :], ess[h][:, kt, :],
                                        mask_sb[:, kt - 4 * qc, :], MULT)
                            drain_pending(3 if su == 0 else 2)
                            if fi < len(fillers) and (b == 0 or kt % 2 == 1):
                                fillers[fi]()
                                fi += 1
                        for h in heads:
                            pending.append(dict(b=b, h=h, qc=qc, es=ess[h],
                                                k=0))
                    while fi < len(fillers):
                        fillers[fi]()
                        fi += 1
                # drain remaining nums + P3(3)
                drain_pending(10 ** 9)
                if dbg:
                    for b in range(B):
                        sl = slice(b * T, (b + 1) * T)
                        nc.sync.dma_start(d_qt.ap()[:, sl], QT[b][:])
                        nc.sync.dma_start(d_kt.ap()[:, sl], KT[b][:])
                        nc.sync.dma_start(d_qt2.ap()[:, sl], QT2[b][:])
                        nc.sync.dma_start(d_kt2.ap()[:, sl], KT2[b][:])
                        n = 8 * NH * 65
                        nc.sync.dma_start(
                            d_vh.ap()[:, b * n:(b + 1) * n],
                            VH[b][:].rearrange("p a h o -> p (a h o)"))
                        nc.sync.dma_start(d_hot.ap()[:, sl], HOT[b][:])
                if 3 in phases:
                    for sp in range(4):
                        for c in p3_pair_closures(3, sp):
                            c()

            if reps == 1:
                emit()
            else:
                with tc.For_i(0, reps, 1):
                    emit()
    nc.compile()
    return nc


def _rne_bf16(a):
    """float32 ndarray -> bfloat16 (round to nearest even), fast path."""
    import ml_dtypes
    v = np.ascontiguousarray(a, dtype=np.float32).view(np.uint32)
    r = ((v >> 16) & 1) + 0x7FFF
    return ((v + r) >> 16).astype(np.uint16).view(ml_dtypes.bfloat16)


def prep_in_maps(x, W_Q, W_K, W_V, W_O, FK0, PK0, FV0, PV0, FK1, PK1, FV1, PV1):
    x = np.asarray(x, dtype=np.float32)
    W_K_eff = np.array(W_K, dtype=np.float32, copy=True)
    W_V_eff = np.array(W_V, dtype=np.float32, copy=True)
    for tier, (FK, PK, FV, PV) in {0: (FK0, PK0, FV0, PV0),
                                   1: (FK1, PK1, FV1, PV1)}.items():
        FK = np.asarray(FK); PK = np.asarray(PK)
        FV = np.asarray(FV); PV = np.asarray(PV)
        lo = IN_OFF[tier + 1]
        for h in range(NHS[tier]):
            col = OUT_OFF[tier] + h * DK
            W_K_eff[lo:, col:col + DK] += FK[:, h * RANK:(h + 1) * RANK] @ PK[h]
            W_V_eff[lo:, col:col + DK] += FV[:, h * RANK:(h + 1) * RANK] @ PV[h]
    W_Q = np.asarray(W_Q, dtype=np.float32)
    W_O = np.asarray(W_O, dtype=np.float32)

    xT_bf = np.ascontiguousarray(_rne_bf16(x.reshape(BT, D)).T)

    k = np.arange(128)[:, None]
    q = np.arange(512)[None, :]
    msk = np.concatenate([(q >= 128 * i + k).astype(np.float32)
                          for i in range(4)] + [np.ones((128, 32), np.float32)],
                         axis=1)
    msk_bf = _rne_bf16(msk)
    cst = np.ones((1, 64), dtype=np.float32)

    in_maps = []
    for c in range(NCORES):
        lo = c * NH * DK
        hi = lo + NH * DK
        wqkc = np.concatenate([W_Q[:, lo:lo + 128], W_K_eff[:, lo:lo + 128],
                               W_Q[:, lo + 128:hi], W_K_eff[:, lo + 128:hi]],
                              axis=1)
        woc = np.zeros((256, D), dtype=np.float32)
        woc[0:128] = W_O[lo:lo + 128]
        woc[128:192] = W_O[lo + 128:hi]
        woc[192:256] = W_O[lo + 128:hi]
        in_maps.append({
            "xT": xT_bf,
            "wqk": _rne_bf16(np.ascontiguousarray(wqkc)),
            "wv": _rne_bf16(np.ascontiguousarray(W_V_eff[:, lo:hi])),
            "wo": _rne_bf16(woc),
            "msk": msk_bf,
            "cst": cst,
        })
    return in_maps


_NC_CACHE = []
_EXEC_CACHE = {}


def get_nc():
    if not _NC_CACHE:
        _NC_CACHE.append(build_nc())
    return _NC_CACHE[0]


def _get_exec(nc, n_cores):
    key = id(nc)
    if key in _EXEC_CACHE:
        return _EXEC_CACHE[key]
    import jax
    from jax.sharding import Mesh, PartitionSpec, NamedSharding
    from jax.experimental.shard_map import shard_map

    bass2jax.install_neuronx_cc_hook()
    partition_name = (nc.partition_id_tensor.name
                      if nc.partition_id_tensor else None)
    in_names, out_names, out_avals, zero_outs = [], [], [], []
    for alloc in nc.m.functions[0].allocations:
        if not isinstance(alloc, mybir.MemoryLocationSet):
            continue
        name = alloc.memorylocations[0].name
        if alloc.kind == "ExternalInput":
            if name != partition_name:
                in_names.append(name)
        elif alloc.kind == "ExternalOutput":
            out_names.append(name)
            shape = tuple(alloc.tensor_shape)
            dtype = mybir.dt.np(alloc.dtype)
            out_avals.append(jax.core.ShapedArray(shape, dtype))
            zero_outs.append(np.zeros(shape, dtype))
    all_in_names = list(in_names) + list(out_names)
    if partition_name is not None:
        all_in_names.append(partition_name)

    def _body(*args):
        operands = list(args)
        if partition_name is not None:
            operands.append(bass2jax.partition_id_tensor())
        outs = bass2jax._bass_exec_p.bind(
            *operands,
            out_avals=tuple(out_avals),
            in_names=tuple(all_in_names),
            out_names=tuple(out_names),
            lowering_input_output_aliases=(),
            sim_require_finite=True,
            sim_require_nnan=True,
            nc=nc,
        )
        return tuple(outs)

    devices = jax.devices()[:n_cores]
    mesh = Mesh(np.asarray(devices), ("core",))
    spec = NamedSharding(mesh, PartitionSpec("core"))
    n_params = len(in_names)
    fn = jax.jit(shard_map(
        _body, mesh=mesh,
        in_specs=(PartitionSpec("core"),) * (n_params + len(out_names)),
        out_specs=(PartitionSpec("core"),) * len(out_names),
        check_rep=False), keep_unused=True)
    entry = dict(fn=fn, in_names=in_names, out_names=out_names,
                 out_avals=out_avals, zero_outs=zero_outs, spec=spec,
                 devices=devices, dev_zeros=None)
    _EXEC_CACHE[key] = entry
    return entry


def _put_sharded(arrs, devices, spec):
    """Put per-core host arrays on devices without a host-side concat."""
    import jax
    shape0 = arrs[0].shape
    global_shape = (len(arrs) * shape0[0],) + tuple(shape0[1:])
    shards = [jax.device_put(a, d) for a, d in zip(arrs, devices)]
    return jax.make_array_from_single_device_arrays(global_shape, spec, shards)


def kernel(**inputs):
    import jax
    nc = get_nc()
    ex = _get_exec(nc, NCORES)
    in_maps = prep_in_maps(**inputs)
    concat_in = [
        _put_sharded([in_maps[c][nm] for c in range(NCORES)],
                     ex["devices"], ex["spec"])
        for nm in ex["in_names"]
    ]
    if ex["dev_zeros"] is None:
        ex["dev_zeros"] = [
            _put_sharded([z] * NCORES, ex["devices"], ex["spec"])
            for z in ex["zero_outs"]
        ]
    outs = ex["fn"](*concat_in, *ex["dev_zeros"])
    jax.block_until_ready(outs)
    o = np.asarray(outs[0]).reshape(NCORES, BT, D)
    acc = o.astype(np.float32).sum(axis=0)
    return acc.reshape(B, T, D)


# revision 5
# speedup vs baseline: 1.9583x; 1.3959x over previous
"""Matryoshka attention Trainium2 kernel v2: 8-core SPMD, head-parallel,
bf16, batch-software-pipelined.

24 heads across 3 tiers -> 3 heads/core; feedback folded into dense
effective K/V projections on host. All operands bf16 (f32 PSUM
accumulation); rel err ~4e-3 vs 2e-2 tolerance.

Per core, per batch b (T=1024 tokens):
  P1(b): project Q^T,K^T (dk-major) and V-hat (token-major + ones col)
         for its 3 heads from a resident bf16 x^T column block.
  P2(b): causal attention, transposed scores S^T = K Q^T, exp on ACT
         (scores bounded, no max subtraction), causal mask via mult on
         Pool (gpsimd), denominator via ones column, normalization via
         K=1 broadcast matmul of the reciprocal row.
  P3(b): partial out stripes = headout^T @ W_O rows, bf16 out DMA;
         host sums the 8 partials.
Emission interleaves P1(b+2)/P1(b+3) chunks and P3(b-1) stripes between
P2(b) score steps so the PE queue never drains; num matmuls of unit j-1
are interleaved kt-by-kt with unit j's score matmuls.
"""

import sys

if "/opt/trn_rl_repo" not in sys.path:
    sys.path.insert(0, "/opt/trn_rl_repo")

import numpy as np

import concourse.bass as bass
import concourse.tile as tile
from concourse import bacc, mybir
from concourse import bass2jax

F32 = mybir.dt.float32
F32R = mybir.dt.float32r
BF = mybir.dt.bfloat16
AF = mybir.ActivationFunctionType
MULT = mybir.AluOpType.mult

B, T, D = 4, 1024, 2048
BT = B * T
DK = 64
NH = 3
NCORES = 8
IN_OFF = [0, 256, 1024, 2048]
OUT_OFF = [0, 256, 768, 1536]
NHS = [4, 8, 12]
RANK = 8
KD = D // 128  # 16 contraction chunks


def build_nc(dbg=False, reps=1, phases=(1, 2, 3)):
    nc = bacc.Bacc("TRN2", target_bir_lowering=False, debug=False)
    xT = nc.dram_tensor("xT", [D, BT], BF, kind="ExternalInput")
    wqk = nc.dram_tensor("wqk", [D, 384], BF, kind="ExternalInput")
    wv = nc.dram_tensor("wv", [D, 192], BF, kind="ExternalInput")
    wo = nc.dram_tensor("wo", [256, D], BF, kind="ExternalInput")
    # msk: cols 0:2048 = 4 causal diagonal masks, cols 2048:2080 = ones
    msk = nc.dram_tensor("msk", [128, 2080], BF, kind="ExternalInput")
    cst = nc.dram_tensor("cst", [1, 64], F32, kind="ExternalInput")
    out = nc.dram_tensor("out", [BT, D], BF, kind="ExternalOutput")
    if dbg:
        d_qt = nc.dram_tensor("d_qt", [128, BT], BF, kind="ExternalOutput")
        d_kt = nc.dram_tensor("d_kt", [128, BT], BF, kind="ExternalOutput")
        d_qt2 = nc.dram_tensor("d_qt2", [64, BT], BF, kind="ExternalOutput")
        d_kt2 = nc.dram_tensor("d_kt2", [128, BT], BF, kind="ExternalOutput")
        d_vh = nc.dram_tensor("d_vh", [128, B * 8 * NH * 65], BF,
                              kind="ExternalOutput")
        d_hot = nc.dram_tensor("d_hot", [128, BT], BF, kind="ExternalOutput")

    with tile.TileContext(nc) as tc:
        with tc.tile_pool(name="pers", bufs=1) as pers, \
             tc.tile_pool(name="xp", bufs=2) as xp, \
             tc.tile_pool(name="esp", bufs=4) as esp, \
             tc.tile_pool(name="recp", bufs=1) as recp, \
             tc.tile_pool(name="osp", bufs=4) as osp, \
             tc.tile_pool(name="qkps", bufs=1, space="PSUM") as qkps, \
             tc.tile_pool(name="sps", bufs=3, space="PSUM") as sps, \
             tc.tile_pool(name="nps", bufs=2, space="PSUM") as nps, \
             tc.tile_pool(name="bp3", bufs=2, space="PSUM") as bp3:
            wqk_sb = pers.tile([128, KD, 384], BF)
            wv_sb = pers.tile([128, KD, 192], BF)
            wo_sb = pers.tile([128, 2, D], BF)
            mask_sb = pers.tile([128, 4, 512], BF)
            ones_sb = pers.tile([1, 64], F32R)
            QT = [pers.tile([128, T], BF, name=f"qt{b}") for b in range(B)]
            KT = [pers.tile([128, T], BF, name=f"kt{b}") for b in range(B)]
            VH = [pers.tile([128, 8, NH, 65], BF, name=f"vh{b}") for b in range(B)]
            # head-2 Q^T/K^T/hoT duplicated in BOTH partition halves so
            # adjacent head-2 score (and P3 stripe) matmuls alternate PE row
            # groups 0/64 and run concurrently
            Q2ALL = [pers.tile([128, T], BF, name=f"q2a{b}") for b in range(B)]
            K2ALL = [pers.tile([128, T], BF, name=f"k2a{b}") for b in range(B)]
            HOT2ALL = [pers.tile([128, T], BF, name=f"h2a{b}")
                       for b in range(B)]
            HOT = [pers.tile([128, T], BF, name=f"hot{b}") for b in range(B)]

            # prelude: first x block interleaved with chunked weight loads so
            # the first matmuls start ~3us in instead of ~17us
            xs00 = pers.tile([128, KD, 512], BF, name="xs00")
            for kc in range(4):
                nc.sync.dma_start(
                    xs00[:, kc * 4:(kc + 1) * 4, :],
                    xT.ap()[kc * 512:(kc + 1) * 512, 0:512].rearrange(
                        "(k p) n -> p k n", p=128))
                nc.sync.dma_start(
                    wqk_sb[:, kc * 4:(kc + 1) * 4, :],
                    wqk.ap()[kc * 512:(kc + 1) * 512, :].rearrange(
                        "(k p) n -> p k n", p=128))
                nc.sync.dma_start(
                    wv_sb[:, kc * 4:(kc + 1) * 4, :],
                    wv.ap()[kc * 512:(kc + 1) * 512, :].rearrange(
                        "(k p) n -> p k n", p=128))
            nc.sync.dma_start(wo_sb[:], wo.ap().rearrange(
                "(k p) n -> p k n", p=128))
            nc.sync.dma_start(mask_sb[:], msk.ap()[:, 0:2048].rearrange(
                "p (i n) -> p i n", i=4))
            nc.sync.dma_start(ones_sb[:], cst.ap().bitcast(F32R))

            def emit():
                for b in range(B):
                    nc.sync.dma_start(
                        VH[b][:, :, :, 64:65],
                        msk.ap()[:, 2048:2072].rearrange(
                            "p (k h o) -> p k h o", k=8, o=1))

                # ---------- P1 chunk closures ----------
                def p1_half_closures(b, half):
                    col = b * T + half * 512
                    st = {}

                    def a_dma():
                        if b == 0 and half == 0:
                            st["xs"] = xs00
                            return
                        xs = xp.tile([128, KD, 512], BF, tag="xs", name="xs")
                        for kc in range(4):
                            nc.sync.dma_start(
                                xs[:, kc * 4:(kc + 1) * 4, :],
                                xT.ap()[kc * 512:(kc + 1) * 512,
                                        col:col + 512].rearrange(
                                    "(k p) n -> p k n", p=128))
                        st["xs"] = xs

                    def a_qk(mt):
                        def f():
                            xs = st["xs"]
                            ps = qkps.tile([128, 512], F32, tag="qk", name="qk")
                            for kd in range(KD):
                                nc.tensor.matmul(
                                    ps[:], wqk_sb[:, kd, mt * 128:(mt + 1) * 128],
                                    xs[:, kd, :], start=kd == 0, stop=kd == KD - 1)
                            dst = half * 512
                            if mt == 0:
                                nc.vector.tensor_copy(
                                    QT[b][:, dst:dst + 512], ps[:])
                            elif mt == 1:
                                nc.scalar.copy(KT[b][:, dst:dst + 512], ps[:])
                            else:
                                nc.vector.tensor_copy(
                                    Q2ALL[b][0:64, dst:dst + 512], ps[0:64, :])
                                nc.scalar.copy(
                                    K2ALL[b][0:64, dst:dst + 512],
                                    ps[64:128, :])
                                nc.vector.tensor_copy(
                                    Q2ALL[b][64:128, dst:dst + 512],
                                    ps[0:64, :])
                                nc.scalar.copy(
                                    K2ALL[b][64:128, dst:dst + 512],
                                    ps[64:128, :])
                        return f

                    def a_v(pr):
                        def f():
                            xs = st["xs"]
                            for s2 in range(2):
                                sub = pr * 2 + s2
                                pv = nps.tile([128, 192], F32, tag="pn",
                                              name="pv")
                                for kd in range(KD):
                                    nc.tensor.matmul(
                                        pv[:],
                                        xs[:, kd, sub * 128:(sub + 1) * 128],
                                        wv_sb[:, kd, :],
                                        start=kd == 0, stop=kd == KD - 1)
                                so = half * 4 + sub
                                nc.vector.tensor_copy(
                                    VH[b][:, so, :, 0:64],
                                    pv[:].rearrange("p (h d) -> p h d", h=NH))
                        return f

                    return [a_dma, a_qk(0), a_v(0), a_qk(1), a_v(1), a_qk(2)]

                # ---------- P3 stripe-pair closures ----------
                # Two adjacent stripes per group: their head-2 K=64 matmuls
                # use PE row groups 64:128 (from KT2) and 0:64 (dup copy in
                # QT2) so they run concurrently.
                def p3_pair_closures(b, sp):
                    s0, s1 = 2 * sp, 2 * sp + 1
                    ls0 = slice(s0 * 128, (s0 + 1) * 128)
                    ls1 = slice(s1 * 128, (s1 + 1) * 128)
                    st = {}

                    def a_nt(nt):
                        def f():
                            if "o0" not in st:
                                st["o0"] = osp.tile([128, D], BF, tag="osb",
                                                    name="osb")
                                st["o1"] = osp.tile([128, D], BF, tag="osb",
                                                    name="osb")
                            ns = slice(nt * 512, (nt + 1) * 512)
                            po_a = bp3.tile([128, 512], F32, tag="bp3",
                                            name="po")
                            po_b = bp3.tile([128, 512], F32, tag="bp3",
                                            name="po")
                            nc.tensor.matmul(po_a[:], HOT[b][:, ls0],
                                             wo_sb[:, 0, ns],
                                             start=True, stop=False)
                            nc.tensor.matmul(po_b[:], HOT[b][:, ls1],
                                             wo_sb[:, 0, ns],
                                             start=True, stop=False)
                            nc.tensor.matmul(po_a[:],
                                             HOT2ALL[b][64:128, ls0],
                                             wo_sb[64:128, 1, ns],
                                             start=False, stop=True)
                            nc.tensor.matmul(po_b[:], HOT2ALL[b][0:64, ls1],
                                             wo_sb[0:64, 1, ns],
                                             start=False, stop=True)
                            if nt % 2 == 0:
                                nc.vector.tensor_copy(st["o0"][:, ns], po_a[:])
                                nc.scalar.copy(st["o1"][:, ns], po_b[:])
                            else:
                                nc.scalar.copy(st["o0"][:, ns], po_a[:])
                                nc.vector.tensor_copy(st["o1"][:, ns], po_b[:])
                            if nt == 3:
                                nc.sync.dma_start(
                                    out.ap()[b * T + s0 * 128:
                                             b * T + (s0 + 1) * 128, :],
                                    st["o0"][:])
                                nc.sync.dma_start(
                                    out.ap()[b * T + s1 * 128:
                                             b * T + (s1 + 1) * 128, :],
                                    st["o1"][:])
                        return f

                    return [a_nt(0), a_nt(1), a_nt(2), a_nt(3)]

                # ---------- P2 helpers ----------
                def k_lhsT(b, h, kt):
                    sl = slice(kt * 128, (kt + 1) * 128)
                    if h == 0:
                        return KT[b][0:64, sl]
                    if h == 1:
                        return KT[b][64:128, sl]
                    if kt % 2 == 0:
                        return K2ALL[b][0:64, sl]
                    return K2ALL[b][64:128, sl]

                def q_rhs(b, h, qc, kt=0):
                    sl = slice(qc * 512, (qc + 1) * 512)
                    if h == 0:
                        return QT[b][0:64, sl]
                    if h == 1:
                        return QT[b][64:128, sl]
                    if kt % 2 == 0:
                        return Q2ALL[b][0:64, sl]
                    return Q2ALL[b][64:128, sl]

                def finalize(b, h, qc, pn):
                    qoff = qc * 512
                    rec = recp.tile([1, 512], F32R, tag="rec", name="rec")
                    with nc.allow_low_precision(
                            reason="softmax denominator reciprocal"):
                        nc.vector.reciprocal(rec[:], pn[64:65, :])
                    pb = bp3.tile([128, 512], F32, tag="bp3", name="pb")
                    nc.tensor.matmul(pb[0:64, :], ones_sb[:], rec[:],
                                     start=True, stop=True)
                    bc = recp.tile([64, 512], F32R, tag="bc", name="bc")
                    nc.vector.tensor_copy(bc[:], pb[0:64, :])
                    if h == 0:
                        dest = HOT[b][0:64, qoff:qoff + 512]
                    elif h == 1:
                        dest = HOT[b][64:128, qoff:qoff + 512]
                    else:
                        dest = HOT2ALL[b][64:128, qoff:qoff + 512]
                    nc.vector.tensor_tensor(dest, pn[0:64, :], bc[:], MULT)
                    if h == 2:
                        # dup copy at partitions 0:64 for P3's odd-stripe
                        # row-group pairing
                        nc.scalar.copy(HOT2ALL[b][0:64, qoff:qoff + 512],
                                       HOT2ALL[b][64:128, qoff:qoff + 512])

                # ---------- pipeline ----------
                if 2 not in phases:
                    for b in range(B):
                        for c in p1_half_closures(b, 0):
                            c()
                        for c in p1_half_closures(b, 1):
                            c()
                    return

                # prologue: P1 of batch 0 only; P1(1)/P1(2) stream in as
                # batch-0 fillers so P2 starts sooner
                for half in (0, 1):
                    for c in p1_half_closures(0, half):
                        c()

                from collections import deque
                pending = deque()  # dicts: b, h, qc, es, pn, k

                def drain_pending(n):
                    """Emit up to n num matmuls from the pending queue."""
                    while n > 0 and pending:
                        p = pending[0]
                        pnkt = 4 * p["qc"] + 4
                        if "pn" not in p:
                            p["pn"] = nps.tile([128, 512], F32, tag="pn",
                                               name="pn")
                        while n > 0 and p["k"] < pnkt:
                            kt = p["k"]
                            nc.tensor.matmul(
                                p["pn"][0:65, :], VH[p["b"]][:, kt, p["h"], :],
                                p["es"][:, kt, :],
                                start=kt == 0, stop=kt == pnkt - 1)
                            p["k"] += 1
                            n -= 1
                        if p["k"] >= pnkt:
                            finalize(p["b"], p["h"], p["qc"], p["pn"])
                            pending.popleft()

                for b in range(B):
                    fillers = []
                    if b == 0:
                        fillers += p1_half_closures(1, 0)
                        fillers += p1_half_closures(1, 1)
                        fillers += p1_half_closures(2, 0)
                        fillers += p1_half_closures(2, 1)
                    elif b == 1:
                        fillers += p1_half_closures(3, 0)
                        if 3 in phases:
                            for sp in range(4):
                                fillers += p3_pair_closures(0, sp)
                    elif b == 2:
                        fillers += p1_half_closures(3, 1)
                        if 3 in phases:
                            for sp in range(4):
                                fillers += p3_pair_closures(1, sp)
                    else:
                        if 3 in phases:
                            for sp in range(4):
                                fillers += p3_pair_closures(2, sp)
                    fi = 0
                    # super-units: heads 0+1 paired (adjacent score matmuls in
                    # PE row groups 0 and 64 run concurrently), head 2 alone
                    for su, qc in [(0, 0), (0, 1), (2, 0), (2, 1)]:
                        nkt = 4 * qc + 4
                        heads = (0, 1) if su == 0 else (2,)
                        ess = {h: esp.tile([128, nkt, 512], BF,
                                           tag=f"es{nkt}", name="es")
                               for h in heads}
                        for step in range(nkt if su == 0 else nkt // 2):
                            kts = ([step] if su == 0
                                   else [2 * step, 2 * step + 1])
                            for kt in kts:
                                for h in heads:
                                    psc = sps.tile([128, 512], F32, tag="psc",
                                                   name="psc")
                                    nc.tensor.matmul(psc[:],
                                                     k_lhsT(b, h, kt),
                                                     q_rhs(b, h, qc, kt),
                                                     start=True, stop=True)
                                    nc.scalar.activation(ess[h][:, kt, :],
                                                         psc[:],
                                                         AF.Exp, scale=0.125)
                                    if kt >= 4 * qc:
                                        nc.gpsimd.tensor_tensor(
                                            ess[h][:, kt, :], ess[h][:, kt, :],
                                            mask_sb[:, kt - 4 * qc, :], MULT)
                            drain_pending(3 if su == 0 else 4)
                            if fi < len(fillers) and (b == 0 or step % 2 == 1
                                                      or su != 0):
                                fillers[fi]()
                                fi += 1
                        for h in heads:
                            pending.append(dict(b=b, h=h, qc=qc, es=ess[h],
                                                k=0))
                    while fi < len(fillers):
                        fillers[fi]()
                        fi += 1
                # drain remaining nums + P3(3)
                drain_pending(10 ** 9)
                if dbg:
                    for b in range(B):
                        sl = slice(b * T, (b + 1) * T)
                        nc.sync.dma_start(d_qt.ap()[:, sl], QT[b][:])
                        nc.sync.dma_start(d_kt.ap()[:, sl], KT[b][:])
                        nc.sync.dma_start(d_qt2.ap()[:, sl],
                                          Q2ALL[b][0:64, :])
                        nc.sync.dma_start(d_kt2.ap()[0:64, sl],
                                          K2ALL[b][0:64, :])
                        nc.sync.dma_start(d_kt2.ap()[64:128, sl],
                                          HOT2ALL[b][64:128, :])
                        n = 8 * NH * 65
                        nc.sync.dma_start(
                            d_vh.ap()[:, b * n:(b + 1) * n],
                            VH[b][:].rearrange("p a h o -> p (a h o)"))
                        nc.sync.dma_start(d_hot.ap()[:, sl], HOT[b][:])
                if 3 in phases:
                    for sp in range(4):
                        for c in p3_pair_closures(3, sp):
                            c()

            if reps == 1:
                emit()
            else:
                with tc.For_i(0, reps, 1):
                    emit()
    nc.compile()
    return nc


def _rne_bf16(a):
    """float32 ndarray -> bfloat16 (round to nearest even), fast path."""
    import ml_dtypes
    v = np.ascontiguousarray(a, dtype=np.float32).view(np.uint32)
    r = ((v >> 16) & 1) + 0x7FFF
    return ((v + r) >> 16).astype(np.uint16).view(ml_dtypes.bfloat16)


def prep_in_maps(x, W_Q, W_K, W_V, W_O, FK0, PK0, FV0, PV0, FK1, PK1, FV1, PV1):
    x = np.asarray(x, dtype=np.float32)
    W_K_eff = np.array(W_K, dtype=np.float32, copy=True)
    W_V_eff = np.array(W_V, dtype=np.float32, copy=True)
    for tier, (FK, PK, FV, PV) in {0: (FK0, PK0, FV0, PV0),
                                   1: (FK1, PK1, FV1, PV1)}.items():
        FK = np.asarray(FK); PK = np.asarray(PK)
        FV = np.asarray(FV); PV = np.asarray(PV)
        lo = IN_OFF[tier + 1]
        for h in range(NHS[tier]):
            col = OUT_OFF[tier] + h * DK
            W_K_eff[lo:, col:col + DK] += FK[:, h * RANK:(h + 1) * RANK] @ PK[h]
            W_V_eff[lo:, col:col + DK] += FV[:, h * RANK:(h + 1) * RANK] @ PV[h]
    W_Q = np.asarray(W_Q, dtype=np.float32)
    W_O = np.asarray(W_O, dtype=np.float32)

    xT_bf = np.ascontiguousarray(_rne_bf16(x.reshape(BT, D)).T)

    k = np.arange(128)[:, None]
    q = np.arange(512)[None, :]
    msk = np.concatenate([(q >= 128 * i + k).astype(np.float32)
                          for i in range(4)] + [np.ones((128, 32), np.float32)],
                         axis=1)
    msk_bf = _rne_bf16(msk)
    cst = np.ones((1, 64), dtype=np.float32)

    in_maps = []
    for c in range(NCORES):
        lo = c * NH * DK
        hi = lo + NH * DK
        wqkc = np.concatenate([W_Q[:, lo:lo + 128], W_K_eff[:, lo:lo + 128],
                               W_Q[:, lo + 128:hi], W_K_eff[:, lo + 128:hi]],
                              axis=1)
        woc = np.zeros((256, D), dtype=np.float32)
        woc[0:128] = W_O[lo:lo + 128]
        woc[128:192] = W_O[lo + 128:hi]
        woc[192:256] = W_O[lo + 128:hi]
        in_maps.append({
            "xT": xT_bf,
            "wqk": _rne_bf16(np.ascontiguousarray(wqkc)),
            "wv": _rne_bf16(np.ascontiguousarray(W_V_eff[:, lo:hi])),
            "wo": _rne_bf16(woc),
            "msk": msk_bf,
            "cst": cst,
        })
    return in_maps


_NC_CACHE = []
_EXEC_CACHE = {}


def get_nc():
    if not _NC_CACHE:
        _NC_CACHE.append(build_nc())
    return _NC_CACHE[0]


def _get_exec(nc, n_cores):
    key = id(nc)
    if key in _EXEC_CACHE:
        return _EXEC_CACHE[key]
    import jax
    from jax.sharding import Mesh, PartitionSpec, NamedSharding
    from jax.experimental.shard_map import shard_map

    bass2jax.install_neuronx_cc_hook()
    partition_name = (nc.partition_id_tensor.name
                      if nc.partition_id_tensor else None)
    in_names, out_names, out_avals, zero_outs = [], [], [], []
    for alloc in nc.m.functions[0].allocations:
        if not isinstance(alloc, mybir.MemoryLocationSet):
            continue
        name = alloc.memorylocations[0].name
        if alloc.kind == "ExternalInput":
            if name != partition_name:
                in_names.append(name)
        elif alloc.kind == "ExternalOutput":
            out_names.append(name)
            shape = tuple(alloc.tensor_shape)
            dtype = mybir.dt.np(alloc.dtype)
            out_avals.append(jax.core.ShapedArray(shape, dtype))
            zero_outs.append(np.zeros(shape, dtype))
    all_in_names = list(in_names) + list(out_names)
    if partition_name is not None:
        all_in_names.append(partition_name)

    def _body(*args):
        operands = list(args)
        if partition_name is not None:
            operands.append(bass2jax.partition_id_tensor())
        outs = bass2jax._bass_exec_p.bind(
            *operands,
            out_avals=tuple(out_avals),
            in_names=tuple(all_in_names),
            out_names=tuple(out_names),
            lowering_input_output_aliases=(),
            sim_require_finite=True,
            sim_require_nnan=True,
            nc=nc,
        )
        return tuple(outs)

    devices = jax.devices()[:n_cores]
    mesh = Mesh(np.asarray(devices), ("core",))
    spec = NamedSharding(mesh, PartitionSpec("core"))
    n_params = len(in_names)
    fn = jax.jit(shard_map(
        _body, mesh=mesh,
        in_specs=(PartitionSpec("core"),) * (n_params + len(out_names)),
        out_specs=(PartitionSpec("core"),) * len(out_names),
        check_rep=False), keep_unused=True)
    entry = dict(fn=fn, in_names=in_names, out_names=out_names,
                 out_avals=out_avals, zero_outs=zero_outs, spec=spec,
                 devices=devices, dev_zeros=None)
    _EXEC_CACHE[key] = entry
    return entry


def _put_sharded(arrs, devices, spec):
    """Put per-core host arrays on devices without a host-side concat."""
    import jax
    shape0 = arrs[0].shape
    global_shape = (len(arrs) * shape0[0],) + tuple(shape0[1:])
    shards = [jax.device_put(a, d) for a, d in zip(arrs, devices)]
    return jax.make_array_from_single_device_arrays(global_shape, spec, shards)


def kernel(**inputs):
    import jax
    nc = get_nc()
    ex = _get_exec(nc, NCORES)
    in_maps = prep_in_maps(**inputs)
    concat_in = [
        _put_sharded([in_maps[c][nm] for c in range(NCORES)],
                     ex["devices"], ex["spec"])
        for nm in ex["in_names"]
    ]
    if ex["dev_zeros"] is None:
        ex["dev_zeros"] = [
            _put_sharded([z] * NCORES, ex["devices"], ex["spec"])
            for z in ex["zero_outs"]
        ]
    outs = ex["fn"](*concat_in, *ex["dev_zeros"])
    jax.block_until_ready(outs)
    o = np.asarray(outs[0]).reshape(NCORES, BT, D)
    acc = o.astype(np.float32).sum(axis=0)
    return acc.reshape(B, T, D)
